# revision 34
# baseline (speedup 1.0000x reference)
import sys

sys.path.insert(0, "/opt/trn_rl_repo")

from contextlib import ExitStack

import numpy as np
import ml_dtypes

B, T, E, D = 4, 4096, 768, 64
P = 128
TQ = T // 2
NQT = TQ // P
NKV = T // P
NPAIR = NKV // 2
EC = E // P
BF16 = ml_dtypes.bfloat16
N_WARM = 12

_CACHE = {}


def _build_bass():
    import concourse.bacc as bacc
    import concourse.mybir as mybir
    import concourse.tile as tile

    nc = bacc.Bacc("TRN2", target_bir_lowering=False)
    f32 = mybir.dt.float32
    bf16 = mybir.dt.bfloat16

    xkv_d = nc.dram_tensor("xkv", (E, T), bf16, kind="ExternalInput")
    xq_d = nc.dram_tensor("xq", (E, TQ), bf16, kind="ExternalInput")
    wqab_d = nc.dram_tensor("wqab", (E, 3 * P), bf16, kind="ExternalInput")
    cmask_d = nc.dram_tensor("cmask", (P, 3 * P), bf16, kind="ExternalInput")
    out_d = nc.dram_tensor("out", (D + 1, TQ), f32, kind="ExternalOutput")

    with ExitStack() as ctx:
        tc = ctx.enter_context(tile.TileContext(nc))
        const = ctx.enter_context(tc.tile_pool(name="const", bufs=1))
        xpool = ctx.enter_context(tc.tile_pool(name="x", bufs=1))
        spool = ctx.enter_context(tc.tile_pool(name="sb", bufs=1))
        ptpool = ctx.enter_context(tc.tile_pool(name="pt", bufs=8))
        obpool = ctx.enter_context(tc.tile_pool(name="ob", bufs=2))
        psc = ctx.enter_context(tc.tile_pool(name="psc", bufs=2, space="PSUM"))
        scr = ctx.enter_context(tc.tile_pool(name="scr", bufs=2, space="PSUM"))
        pout = ctx.enter_context(tc.tile_pool(name="pout", bufs=2, space="PSUM"))

        scratch = const.tile([P, 512], bf16)
        nc.vector.memset(scratch[:], 1.0)
        for wi in range(N_WARM):
            pw = scr.tile([P, 512], f32, tag="scr", name=f"warm{wi}")
            nc.tensor.matmul(
                pw[:, :], lhsT=scratch[:, 0:P], rhs=scratch[:],
                start=True, stop=True,
            )
            if wi == N_WARM - 1:
                nc.vector.tensor_copy(scratch[0:1, 0:1], pw[0:1, 0:1])

        wqab_t = const.tile([P, EC * 3 * P], bf16)
        nc.sync.dma_start(
            out=wqab_t.rearrange("p (ec d) -> p ec d", d=3 * P),
            in_=wqab_d.rearrange("(ec p) d -> p ec d", p=P),
        )
        cmask_t = const.tile([P, 3 * P], bf16)
        nc.sync.dma_start(out=cmask_t[:], in_=cmask_d[:])
        mprev_t = cmask_t[:, 0:P]
        mlast_t = cmask_t[:, P:2 * P]
        ident_t = cmask_t[:, 2 * P:3 * P]
        xq_t = xpool.tile([P, EC * TQ], bf16)
        xkv_t = xpool.tile([P, EC * T], bf16)

        def dma_xq(j0, j1):
            nc.sync.dma_start(
                out=xq_t.rearrange("p (ec t) -> p ec t", t=TQ)[:, :, j0 * 512:j1 * 512],
                in_=xq_d.rearrange("(ec p) t -> p ec t", p=P)[:, :, j0 * 512:j1 * 512],
            )

        def dma_xkv(j0, j1):
            nc.sync.dma_start(
                out=xkv_t.rearrange("p (ec t) -> p ec t", t=T)[:, :, j0 * 512:j1 * 512],
                in_=xkv_d.rearrange("(ec p) t -> p ec t", p=P)[:, :, j0 * 512:j1 * 512],
            )

        dma_xq(0, 1)
        dma_xkv(0, 1)
        dma_xkv(1, 2)
        dma_xq(1, 2)
        dma_xkv(2, 4)
        dma_xq(2, 3)
        dma_xkv(4, 6)
        dma_xq(3, 4)
        dma_xkv(6, 8)

        qT_t = spool.tile([P, TQ], bf16)
        kT2_t = spool.tile([P, NPAIR * P], bf16)
        vstage = spool.tile([P, NPAIR * P], bf16)
        v1_t = spool.tile([P, NKV * (D + 1)], bf16)
        nc.vector.memset(v1_t[:], 1.0)

        xkv_v = xkv_t.rearrange(
            "p (ec ch pp par g) -> p ec ch par pp g", ec=EC, ch=8, pp=2, par=2, g=P
        )

        qt_ps = {}

        def qt_proj_a(j):
            ps = scr.tile([P, 512], f32, tag="scr", name=f"psq{j}")
            qt_ps[j] = ps
            for ec in range(3):
                nc.tensor.matmul(
                    ps[:, :],
                    lhsT=wqab_t[:, ec * 3 * P: ec * 3 * P + P],
                    rhs=xq_t[:, ec * TQ + j * 512: ec * TQ + (j + 1) * 512],
                    start=(ec == 0),
                    stop=False,
                )

        def qt_proj_b(j):
            ps = qt_ps[j]
            for ec in range(3, EC):
                nc.tensor.matmul(
                    ps[:, :],
                    lhsT=wqab_t[:, ec * 3 * P: ec * 3 * P + P],
                    rhs=xq_t[:, ec * TQ + j * 512: ec * TQ + (j + 1) * 512],
                    start=False,
                    stop=(ec == EC - 1),
                )
            nc.vector.tensor_copy(qT_t[:, j * 512:(j + 1) * 512], ps[:, :])

        def qt_proj(j):
            qt_proj_a(j)
            qt_proj_b(j)

        def kv_projA(c):
            psA = scr.tile([P, 512], f32, tag="scr", name=f"pskA{c}")
            for ec in range(EC):
                nc.tensor.matmul(
                    psA[:, 0:256],
                    lhsT=wqab_t[:, ec * 3 * P + P: ec * 3 * P + 2 * P],
                    rhs=xkv_v[:, ec:ec + 1, c:c + 1, 0:1, :, :],
                    start=(ec == 0),
                    stop=(ec == EC - 1),
                )
            blk = slice(2 * c * P, (2 * c + 2) * P)
            nc.vector.tensor_copy(kT2_t[0:D, blk], psA[0:D, 0:256])
            nc.vector.tensor_copy(vstage[D:2 * D, blk], psA[D:2 * D, 0:256])

        def kv_projB(c):
            psB = scr.tile([P, 512], f32, tag="scr", name=f"pskB{c}")
            for ec in range(EC):
                nc.tensor.matmul(
                    psB[:, 0:256],
                    lhsT=wqab_t[:, ec * 3 * P + 2 * P: ec * 3 * P + 3 * P],
                    rhs=xkv_v[:, ec:ec + 1, c:c + 1, 1:2, :, :],
                    start=(ec == 0),
                    stop=(ec == EC - 1),
                )
            blk = slice(2 * c * P, (2 * c + 2) * P)
            nc.vector.tensor_copy(kT2_t[D:2 * D, blk], psB[D:2 * D, 0:256])
            nc.vector.tensor_copy(vstage[0:D, blk], psB[0:D, 0:256])

        def v_trans1(tp):
            pv = scr.tile([P, 512], bf16, tag="scr", name=f"pv{tp}")
            nc.tensor.transpose(
                pv[:, 0:P],
                in_=vstage[:, tp * P:(tp + 1) * P],
                identity=ident_t,
            )
            v1v = v1_t.rearrange("p (k e) -> p k e", e=D + 1)
            nc.vector.tensor_copy(v1v[:, 2 * tp:2 * tp + 1, 0:D], pv[:, D:P])
            nc.vector.tensor_copy(v1v[:, 2 * tp + 1:2 * tp + 2, 0:D], pv[:, 0:D])

        outp_tiles = {}
        pending = []

        def attn_flush():
            if not pending:
                return
            g, t, pt, w, cs = pending.pop(0)
            lo = g * 512
            outp = outp_tiles[g]
            ka, kb = 2 * t, 2 * t + 1
            nc.tensor.matmul(
                outp[:, cs - lo: 512],
                lhsT=v1_t[:, ka * (D + 1):(ka + 1) * (D + 1)],
                rhs=pt[:, 0:w],
                start=(ka == 0),
                stop=(ka == 8 * g + 7),
            )
            nc.tensor.matmul(
                outp[:, cs - lo: 512],
                lhsT=v1_t[:, kb * (D + 1):(kb + 1) * (D + 1)],
                rhs=pt[:, 512:512 + w],
                start=(kb == 0),
                stop=(kb == 8 * g + 7),
            )
            if kb == 8 * g + 7:
                ob = obpool.tile([D + 1, 512], f32)
                nc.vector.tensor_copy(ob[:], outp[:, 0:512])
                nc.sync.dma_start(out=out_d[:, lo: lo + 512], in_=ob[:])

        def attn_pair(g, t):
            lo = g * 512
            if g not in outp_tiles:
                outp_tiles[g] = pout.tile(
                    [D + 1, 512], f32, tag="out", name=f"outp{g}"
                )
            qs = t * P
            cs = max(qs, lo)
            w = lo + 512 - cs
            s = psc.tile([P, 1024], f32, tag="sc", name=f"s{g}_{t}")
            nc.tensor.matmul(
                s[:, 0:w],
                lhsT=kT2_t[0:D, t * P:(t + 1) * P],
                rhs=qT_t[0:D, cs: cs + w],
                start=True,
                stop=True,
            )
            nc.tensor.matmul(
                s[:, 512:512 + w],
                lhsT=kT2_t[D:2 * D, t * P:(t + 1) * P],
                rhs=qT_t[D:2 * D, cs: cs + w],
                start=True,
                stop=True,
            )
            pt = ptpool.tile([P, 1024], bf16)
            nc.scalar.activation(
                pt.rearrange("p (u c) -> p u c", c=512)[:, :, 0:w],
                s.rearrange("p (u c) -> p u c", c=512)[:, :, 0:w],
                func=mybir.ActivationFunctionType.Exp, scale=0.125,
            )
            if cs == qs:
                nc.vector.tensor_mul(pt[:, 0:P], pt[:, 0:P], mprev_t)
                nc.vector.tensor_mul(pt[:, 512:512 + P], pt[:, 512:512 + P], mlast_t)
            pending.append((g, t, pt, w, cs))
            while len(pending) > 3:
                attn_flush()

        qt_proj(0)
        kv_projA(0)
        kv_projB(0)
        v_trans1(0)
        v_trans1(1)
        attn_pair(0, 0)
        kv_projA(1)
        attn_pair(0, 1)
        kv_projB(1)
        qt_proj_a(1)
        attn_pair(0, 2)
        qt_proj_b(1)
        v_trans1(2)
        attn_pair(0, 3)
        v_trans1(3)
        attn_pair(1, 0)
        kv_projA(2)
        attn_pair(1, 1)
        kv_projB(2)
        qt_proj_a(2)
        attn_pair(1, 2)
        qt_proj_b(2)
        attn_pair(1, 3)
        v_trans1(4)
        attn_pair(1, 4)
        kv_projA(3)
        v_trans1(5)
        attn_pair(1, 5)
        kv_projB(3)
        attn_pair(1, 6)
        v_trans1(6)
        attn_pair(1, 7)
        v_trans1(7)
        attn_pair(2, 0)
        kv_projA(4)
        attn_pair(2, 1)
        kv_projB(4)
        attn_pair(2, 2)
        qt_proj_a(3)
        attn_pair(2, 3)
        qt_proj_b(3)
        attn_pair(2, 4)
        v_trans1(8)
        attn_pair(2, 5)
        kv_projA(5)
        attn_pair(2, 6)
        kv_projB(5)
        attn_pair(2, 7)
        v_trans1(9)
        attn_pair(2, 8)
        v_trans1(10)
        attn_pair(2, 9)
        v_trans1(11)
        attn_pair(2, 10)
        attn_pair(2, 11)
        attn_pair(3, 0)
        kv_projA(6)
        attn_pair(3, 1)
        kv_projB(6)
        attn_pair(3, 2)
        v_trans1(12)
        attn_pair(3, 3)
        kv_projA(7)
        attn_pair(3, 4)
        kv_projB(7)
        attn_pair(3, 5)
        v_trans1(13)
        attn_pair(3, 6)
        v_trans1(14)
        attn_pair(3, 7)
        v_trans1(15)
        for t in range(8, NPAIR):
            attn_pair(3, t)
        while pending:
            attn_flush()

    nc.compile()
    return nc


def _shard_inputs(x, Wq, Wk, Wv):
    x = np.asarray(x, np.float32)
    wq = np.asarray(Wq, np.float32)
    wk = np.asarray(Wk, np.float32)
    wv = np.asarray(Wv, np.float32)
    wqab = np.concatenate([wq, wq, wk, wv, wv, wk], axis=1).astype(BF16)
    ident = np.eye(P, dtype=BF16)
    tri = (np.arange(P)[:, None] <= np.arange(P)[None, :]).astype(BF16)
    ones = np.ones((P, P), BF16)
    zeros = np.zeros((P, P), BF16)
    qidx = {h: np.concatenate([np.arange(P) + (2 * i + h) * P for i in range(NQT)]) for h in (0, 1)}
    in_maps = []
    for c in range(8):
        b, h = c // 2, c % 2
        xT = np.ascontiguousarray(x[b].T).astype(BF16)
        xq = np.ascontiguousarray(xT[:, qidx[h]])
        cmask = np.concatenate(
            [tri if h == 0 else ones, zeros if h == 0 else tri, ident], axis=1
        )
        in_maps.append({
            "xkv": xT,
            "xq": xq,
            "wqab": wqab,
            "cmask": cmask,
        })
    return in_maps


def _unshard(results):
    out = np.zeros((B, T, D), np.float32)
    for c, om in enumerate(results):
        b, h = c // 2, c % 2
        o = np.asarray(om["out"], np.float32)
        on = (o[:D] / o[D:D + 1]).T
        for i in range(NQT):
            out[b, (2 * i + h) * P:(2 * i + h + 1) * P] = on[i * P:(i + 1) * P]
    return out


def kernel(x, Wq, Wk, Wv):
    from concourse import bass_utils

    if "nc" not in _CACHE:
        _CACHE["nc"] = _build_bass()
    nc = _CACHE["nc"]
    in_maps = _shard_inputs(x, Wq, Wk, Wv)
    res = bass_utils.run_bass_kernel_spmd(nc, in_maps, core_ids=list(range(8)))
    _CACHE["last_result"] = res
    return _unshard(res.results)


# revision 35
# speedup vs baseline: 1.0299x; 1.0299x over previous
import sys

sys.path.insert(0, "/opt/trn_rl_repo")

from contextlib import ExitStack

import numpy as np
import ml_dtypes

B, T, E, D = 4, 4096, 768, 64
P = 128
TQ = T // 2
NQT = TQ // P
NKV = T // P
NPAIR = NKV // 2
EC = E // P
BF16 = ml_dtypes.bfloat16
N_WARM = 13

_CACHE = {}


def _build_bass():
    import concourse.bacc as bacc
    import concourse.mybir as mybir
    import concourse.tile as tile

    nc = bacc.Bacc("TRN2", target_bir_lowering=False)
    f32 = mybir.dt.float32
    bf16 = mybir.dt.bfloat16

    xkv_d = nc.dram_tensor("xkv", (E, T), bf16, kind="ExternalInput")
    xq_d = nc.dram_tensor("xq", (E, TQ), bf16, kind="ExternalInput")
    wqab_d = nc.dram_tensor("wqab", (E, 3 * P), bf16, kind="ExternalInput")
    cmask_d = nc.dram_tensor("cmask", (P, 3 * P), bf16, kind="ExternalInput")
    out_d = nc.dram_tensor("out", (D + 1, TQ), f32, kind="ExternalOutput")

    with ExitStack() as ctx:
        tc = ctx.enter_context(tile.TileContext(nc))
        const = ctx.enter_context(tc.tile_pool(name="const", bufs=1))
        xpool = ctx.enter_context(tc.tile_pool(name="x", bufs=1))
        spool = ctx.enter_context(tc.tile_pool(name="sb", bufs=1))
        ptpool = ctx.enter_context(tc.tile_pool(name="pt", bufs=8))
        obpool = ctx.enter_context(tc.tile_pool(name="ob", bufs=2))
        psc = ctx.enter_context(tc.tile_pool(name="psc", bufs=2, space="PSUM"))
        scr = ctx.enter_context(tc.tile_pool(name="scr", bufs=2, space="PSUM"))
        pout = ctx.enter_context(tc.tile_pool(name="pout", bufs=2, space="PSUM"))

        scratch = const.tile([P, 512], bf16)
        nc.vector.memset(scratch[:], 1.0)
        for wi in range(N_WARM):
            pw = scr.tile([P, 512], f32, tag="scr", name=f"warm{wi}")
            nc.tensor.matmul(
                pw[:, :], lhsT=scratch[:, 0:P], rhs=scratch[:],
                start=True, stop=True,
            )
            if wi == N_WARM - 1:
                nc.vector.tensor_copy(scratch[0:1, 0:1], pw[0:1, 0:1])

        wqab_t = const.tile([P, EC * 3 * P], bf16)
        nc.sync.dma_start(
            out=wqab_t.rearrange("p (ec d) -> p ec d", d=3 * P),
            in_=wqab_d.rearrange("(ec p) d -> p ec d", p=P),
        )
        cmask_t = const.tile([P, 3 * P], bf16)
        nc.sync.dma_start(out=cmask_t[:], in_=cmask_d[:])
        mprev_t = cmask_t[:, 0:P]
        mlast_t = cmask_t[:, P:2 * P]
        ident_t = cmask_t[:, 2 * P:3 * P]
        xq_t = xpool.tile([P, EC * TQ], bf16)
        xkv_t = xpool.tile([P, EC * T], bf16)

        def dma_xq(j0, j1):
            nc.sync.dma_start(
                out=xq_t.rearrange("p (ec t) -> p ec t", t=TQ)[:, :, j0 * 512:j1 * 512],
                in_=xq_d.rearrange("(ec p) t -> p ec t", p=P)[:, :, j0 * 512:j1 * 512],
            )

        def dma_xkv(j0, j1):
            nc.sync.dma_start(
                out=xkv_t.rearrange("p (ec t) -> p ec t", t=T)[:, :, j0 * 512:j1 * 512],
                in_=xkv_d.rearrange("(ec p) t -> p ec t", p=P)[:, :, j0 * 512:j1 * 512],
            )

        dma_xq(0, 1)
        dma_xkv(0, 1)
        dma_xkv(1, 2)
        dma_xq(1, 2)
        dma_xkv(2, 4)
        dma_xq(2, 3)
        dma_xkv(4, 6)
        dma_xq(3, 4)
        dma_xkv(6, 8)

        qT_t = spool.tile([P, TQ], bf16)
        kT2_t = spool.tile([P, NPAIR * P], bf16)
        vstage = spool.tile([P, NPAIR * P], bf16)
        v1_t = spool.tile([P, NKV * (D + 1)], bf16)
        nc.vector.memset(v1_t[:], 1.0)

        xkv_v = xkv_t.rearrange(
            "p (ec ch pp par g) -> p ec ch par pp g", ec=EC, ch=8, pp=2, par=2, g=P
        )

        qt_ps = {}

        def qt_proj_a(j):
            ps = scr.tile([P, 512], f32, tag="scr", name=f"psq{j}")
            qt_ps[j] = ps
            for ec in range(3):
                nc.tensor.matmul(
                    ps[:, :],
                    lhsT=wqab_t[:, ec * 3 * P: ec * 3 * P + P],
                    rhs=xq_t[:, ec * TQ + j * 512: ec * TQ + (j + 1) * 512],
                    start=(ec == 0),
                    stop=False,
                )

        def qt_proj_b(j):
            ps = qt_ps[j]
            for ec in range(3, EC):
                nc.tensor.matmul(
                    ps[:, :],
                    lhsT=wqab_t[:, ec * 3 * P: ec * 3 * P + P],
                    rhs=xq_t[:, ec * TQ + j * 512: ec * TQ + (j + 1) * 512],
                    start=False,
                    stop=(ec == EC - 1),
                )
            nc.vector.tensor_copy(qT_t[:, j * 512:(j + 1) * 512], ps[:, :])

        def qt_proj(j):
            qt_proj_a(j)
            qt_proj_b(j)

        def kv_projA(c):
            psA = scr.tile([P, 512], f32, tag="scr", name=f"pskA{c}")
            for ec in range(EC):
                nc.tensor.matmul(
                    psA[:, 0:256],
                    lhsT=wqab_t[:, ec * 3 * P + P: ec * 3 * P + 2 * P],
                    rhs=xkv_v[:, ec:ec + 1, c:c + 1, 0:1, :, :],
                    start=(ec == 0),
                    stop=(ec == EC - 1),
                )
            blk = slice(2 * c * P, (2 * c + 2) * P)
            nc.vector.tensor_copy(kT2_t[0:D, blk], psA[0:D, 0:256])
            nc.vector.tensor_copy(vstage[D:2 * D, blk], psA[D:2 * D, 0:256])

        def kv_projB(c):
            psB = scr.tile([P, 512], f32, tag="scr", name=f"pskB{c}")
            for ec in range(EC):
                nc.tensor.matmul(
                    psB[:, 0:256],
                    lhsT=wqab_t[:, ec * 3 * P + 2 * P: ec * 3 * P + 3 * P],
                    rhs=xkv_v[:, ec:ec + 1, c:c + 1, 1:2, :, :],
                    start=(ec == 0),
                    stop=(ec == EC - 1),
                )
            blk = slice(2 * c * P, (2 * c + 2) * P)
            nc.vector.tensor_copy(kT2_t[D:2 * D, blk], psB[D:2 * D, 0:256])
            nc.vector.tensor_copy(vstage[0:D, blk], psB[0:D, 0:256])

        def v_trans1(tp):
            pv = scr.tile([P, 512], bf16, tag="scr", name=f"pv{tp}")
            nc.tensor.transpose(
                pv[:, 0:P],
                in_=vstage[:, tp * P:(tp + 1) * P],
                identity=ident_t,
            )
            v1v = v1_t.rearrange("p (k e) -> p k e", e=D + 1)
            nc.vector.tensor_copy(v1v[:, 2 * tp:2 * tp + 1, 0:D], pv[:, D:P])
            nc.vector.tensor_copy(v1v[:, 2 * tp + 1:2 * tp + 2, 0:D], pv[:, 0:D])

        outp_tiles = {}
        pending = []

        def attn_flush():
            if not pending:
                return
            g, t, pt, w, cs = pending.pop(0)
            lo = g * 512
            outp = outp_tiles[g]
            ka, kb = 2 * t, 2 * t + 1
            nc.tensor.matmul(
                outp[:, cs - lo: 512],
                lhsT=v1_t[:, ka * (D + 1):(ka + 1) * (D + 1)],
                rhs=pt[:, 0:w],
                start=(ka == 0),
                stop=(ka == 8 * g + 7),
            )
            nc.tensor.matmul(
                outp[:, cs - lo: 512],
                lhsT=v1_t[:, kb * (D + 1):(kb + 1) * (D + 1)],
                rhs=pt[:, 512:512 + w],
                start=(kb == 0),
                stop=(kb == 8 * g + 7),
            )
            if kb == 8 * g + 7:
                ob = obpool.tile([D + 1, 512], f32)
                nc.vector.tensor_copy(ob[:], outp[:, 0:512])
                nc.sync.dma_start(out=out_d[:, lo: lo + 512], in_=ob[:])

        def attn_pair(g, t):
            lo = g * 512
            if g not in outp_tiles:
                outp_tiles[g] = pout.tile(
                    [D + 1, 512], f32, tag="out", name=f"outp{g}"
                )
            qs = t * P
            cs = max(qs, lo)
            w = lo + 512 - cs
            s = psc.tile([P, 1024], f32, tag="sc", name=f"s{g}_{t}")
            nc.tensor.matmul(
                s[:, 0:w],
                lhsT=kT2_t[0:D, t * P:(t + 1) * P],
                rhs=qT_t[0:D, cs: cs + w],
                start=True,
                stop=True,
            )
            nc.tensor.matmul(
                s[:, 512:512 + w],
                lhsT=kT2_t[D:2 * D, t * P:(t + 1) * P],
                rhs=qT_t[D:2 * D, cs: cs + w],
                start=True,
                stop=True,
            )
            pt = ptpool.tile([P, 1024], bf16)
            nc.scalar.activation(
                pt.rearrange("p (u c) -> p u c", c=512)[:, :, 0:w],
                s.rearrange("p (u c) -> p u c", c=512)[:, :, 0:w],
                func=mybir.ActivationFunctionType.Exp, scale=0.125,
            )
            if cs == qs:
                nc.vector.tensor_mul(pt[:, 0:P], pt[:, 0:P], mprev_t)
                nc.vector.tensor_mul(pt[:, 512:512 + P], pt[:, 512:512 + P], mlast_t)
            pending.append((g, t, pt, w, cs))
            while len(pending) > 3:
                attn_flush()

        qt_proj(0)
        kv_projA(0)
        kv_projB(0)
        v_trans1(0)
        v_trans1(1)
        attn_pair(0, 0)
        kv_projA(1)
        attn_pair(0, 1)
        kv_projB(1)
        qt_proj_a(1)
        attn_pair(0, 2)
        qt_proj_b(1)
        v_trans1(2)
        attn_pair(0, 3)
        v_trans1(3)
        attn_pair(1, 0)
        kv_projA(2)
        attn_pair(1, 1)
        kv_projB(2)
        qt_proj_a(2)
        attn_pair(1, 2)
        qt_proj_b(2)
        attn_pair(1, 3)
        v_trans1(4)
        attn_pair(1, 4)
        kv_projA(3)
        kv_projB(3)
        attn_pair(1, 5)
        v_trans1(5)
        attn_pair(1, 6)
        v_trans1(6)
        attn_pair(1, 7)
        v_trans1(7)
        attn_pair(2, 0)
        kv_projA(4)
        attn_pair(2, 1)
        kv_projB(4)
        attn_pair(2, 2)
        qt_proj_a(3)
        attn_pair(2, 3)
        qt_proj_b(3)
        attn_pair(2, 4)
        v_trans1(8)
        attn_pair(2, 5)
        kv_projA(5)
        kv_projB(5)
        attn_pair(2, 6)
        attn_pair(2, 7)
        v_trans1(9)
        attn_pair(2, 8)
        v_trans1(10)
        attn_pair(2, 9)
        v_trans1(11)
        attn_pair(2, 10)
        attn_pair(2, 11)
        attn_pair(3, 0)
        kv_projA(6)
        attn_pair(3, 1)
        kv_projB(6)
        attn_pair(3, 2)
        v_trans1(12)
        attn_pair(3, 3)
        kv_projA(7)
        attn_pair(3, 4)
        kv_projB(7)
        attn_pair(3, 5)
        v_trans1(13)
        attn_pair(3, 6)
        v_trans1(14)
        attn_pair(3, 7)
        v_trans1(15)
        for t in range(8, NPAIR):
            attn_pair(3, t)
        while pending:
            attn_flush()

    nc.compile()
    return nc


def _shard_inputs(x, Wq, Wk, Wv):
    x = np.asarray(x, np.float32)
    wq = np.asarray(Wq, np.float32)
    wk = np.asarray(Wk, np.float32)
    wv = np.asarray(Wv, np.float32)
    wqab = np.concatenate([wq, wq, wk, wv, wv, wk], axis=1).astype(BF16)
    ident = np.eye(P, dtype=BF16)
    tri = (np.arange(P)[:, None] <= np.arange(P)[None, :]).astype(BF16)
    ones = np.ones((P, P), BF16)
    zeros = np.zeros((P, P), BF16)
    qidx = {h: np.concatenate([np.arange(P) + (2 * i + h) * P for i in range(NQT)]) for h in (0, 1)}
    in_maps = []
    for c in range(8):
        b, h = c // 2, c % 2
        xT = np.ascontiguousarray(x[b].T).astype(BF16)
        xq = np.ascontiguousarray(xT[:, qidx[h]])
        cmask = np.concatenate(
            [tri if h == 0 else ones, zeros if h == 0 else tri, ident], axis=1
        )
        in_maps.append({
            "xkv": xT,
            "xq": xq,
            "wqab": wqab,
            "cmask": cmask,
        })
    return in_maps


def _unshard(results):
    out = np.zeros((B, T, D), np.float32)
    for c, om in enumerate(results):
        b, h = c // 2, c % 2
        o = np.asarray(om["out"], np.float32)
        on = (o[:D] / o[D:D + 1]).T
        for i in range(NQT):
            out[b, (2 * i + h) * P:(2 * i + h + 1) * P] = on[i * P:(i + 1) * P]
    return out


def kernel(x, Wq, Wk, Wv):
    from concourse import bass_utils

    if "nc" not in _CACHE:
        _CACHE["nc"] = _build_bass()
    nc = _CACHE["nc"]
    in_maps = _shard_inputs(x, Wq, Wk, Wv)
    res = bass_utils.run_bass_kernel_spmd(nc, in_maps, core_ids=list(range(8)))
    _CACHE["last_result"] = res
    return _unshard(res.results)


# revision 36
# speedup vs baseline: 1.0445x; 1.0142x over previous
import sys

sys.path.insert(0, "/opt/trn_rl_repo")

from contextlib import ExitStack

import numpy as np
import ml_dtypes

B, T, E, D = 4, 4096, 768, 64
P = 128
TQ = T // 2
NQT = TQ // P
NKV = T // P
NPAIR = NKV // 2
EC = E // P
BF16 = ml_dtypes.bfloat16
N_WARM = 13

_CACHE = {}


def _build_bass():
    import concourse.bacc as bacc
    import concourse.mybir as mybir
    import concourse.tile as tile

    nc = bacc.Bacc("TRN2", target_bir_lowering=False)
    f32 = mybir.dt.float32
    bf16 = mybir.dt.bfloat16

    xkv_d = nc.dram_tensor("xkv", (E, T), bf16, kind="ExternalInput")
    xq_d = nc.dram_tensor("xq", (E, TQ), bf16, kind="ExternalInput")
    wqab_d = nc.dram_tensor("wqab", (E, 3 * P), bf16, kind="ExternalInput")
    cmask_d = nc.dram_tensor("cmask", (P, 3 * P), bf16, kind="ExternalInput")
    out_d = nc.dram_tensor("out", (D + 1, TQ), f32, kind="ExternalOutput")

    with ExitStack() as ctx:
        tc = ctx.enter_context(tile.TileContext(nc))
        const = ctx.enter_context(tc.tile_pool(name="const", bufs=1))
        xpool = ctx.enter_context(tc.tile_pool(name="x", bufs=1))
        spool = ctx.enter_context(tc.tile_pool(name="sb", bufs=1))
        ptpool = ctx.enter_context(tc.tile_pool(name="pt", bufs=8))
        obpool = ctx.enter_context(tc.tile_pool(name="ob", bufs=2))
        psc = ctx.enter_context(tc.tile_pool(name="psc", bufs=2, space="PSUM"))
        scr = ctx.enter_context(tc.tile_pool(name="scr", bufs=2, space="PSUM"))
        pout = ctx.enter_context(tc.tile_pool(name="pout", bufs=2, space="PSUM"))

        scratch = const.tile([P, 512], bf16)
        nc.any.memset(scratch[:], 1.0)
        for wi in range(N_WARM):
            pw = scr.tile([P, 512], f32, tag="scr", name=f"warm{wi}")
            nc.tensor.matmul(
                pw[:, :], lhsT=scratch[:, 0:P], rhs=scratch[:],
                start=True, stop=True,
            )
            if wi == N_WARM - 1:
                nc.any.tensor_copy(scratch[0:1, 0:1], pw[0:1, 0:1])

        wqab_t = const.tile([P, EC * 3 * P], bf16)
        nc.sync.dma_start(
            out=wqab_t.rearrange("p (ec d) -> p ec d", d=3 * P),
            in_=wqab_d.rearrange("(ec p) d -> p ec d", p=P),
        )
        cmask_t = const.tile([P, 3 * P], bf16)
        nc.sync.dma_start(out=cmask_t[:], in_=cmask_d[:])
        mprev_t = cmask_t[:, 0:P]
        mlast_t = cmask_t[:, P:2 * P]
        ident_t = cmask_t[:, 2 * P:3 * P]
        xq_t = xpool.tile([P, EC * TQ], bf16)
        xkv_t = xpool.tile([P, EC * T], bf16)

        def dma_xq(j0, j1):
            nc.sync.dma_start(
                out=xq_t.rearrange("p (ec t) -> p ec t", t=TQ)[:, :, j0 * 512:j1 * 512],
                in_=xq_d.rearrange("(ec p) t -> p ec t", p=P)[:, :, j0 * 512:j1 * 512],
            )

        def dma_xkv(j0, j1):
            nc.sync.dma_start(
                out=xkv_t.rearrange("p (ec t) -> p ec t", t=T)[:, :, j0 * 512:j1 * 512],
                in_=xkv_d.rearrange("(ec p) t -> p ec t", p=P)[:, :, j0 * 512:j1 * 512],
            )

        dma_xq(0, 1)
        dma_xkv(0, 1)
        dma_xkv(1, 2)
        dma_xq(1, 2)
        dma_xkv(2, 4)
        dma_xq(2, 3)
        dma_xkv(4, 6)
        dma_xq(3, 4)
        dma_xkv(6, 8)

        qT_t = spool.tile([P, TQ], bf16)
        kT2_t = spool.tile([P, NPAIR * P], bf16)
        vstage = spool.tile([P, NPAIR * P], bf16)
        v1_t = spool.tile([P, NKV * (D + 1)], bf16)
        nc.any.memset(v1_t[:], 1.0)

        xkv_v = xkv_t.rearrange(
            "p (ec ch pp par g) -> p ec ch par pp g", ec=EC, ch=8, pp=2, par=2, g=P
        )

        qt_ps = {}

        def qt_proj_a(j):
            ps = scr.tile([P, 512], f32, tag="scr", name=f"psq{j}")
            qt_ps[j] = ps
            for ec in range(3):
                nc.tensor.matmul(
                    ps[:, :],
                    lhsT=wqab_t[:, ec * 3 * P: ec * 3 * P + P],
                    rhs=xq_t[:, ec * TQ + j * 512: ec * TQ + (j + 1) * 512],
                    start=(ec == 0),
                    stop=False,
                )

        def qt_proj_b(j):
            ps = qt_ps[j]
            for ec in range(3, EC):
                nc.tensor.matmul(
                    ps[:, :],
                    lhsT=wqab_t[:, ec * 3 * P: ec * 3 * P + P],
                    rhs=xq_t[:, ec * TQ + j * 512: ec * TQ + (j + 1) * 512],
                    start=False,
                    stop=(ec == EC - 1),
                )
            nc.any.tensor_copy(qT_t[:, j * 512:(j + 1) * 512], ps[:, :])

        def qt_proj(j):
            qt_proj_a(j)
            qt_proj_b(j)

        def kv_projA(c):
            psA = scr.tile([P, 512], f32, tag="scr", name=f"pskA{c}")
            for ec in range(EC):
                nc.tensor.matmul(
                    psA[:, 0:256],
                    lhsT=wqab_t[:, ec * 3 * P + P: ec * 3 * P + 2 * P],
                    rhs=xkv_v[:, ec:ec + 1, c:c + 1, 0:1, :, :],
                    start=(ec == 0),
                    stop=(ec == EC - 1),
                )
            blk = slice(2 * c * P, (2 * c + 2) * P)
            nc.any.tensor_copy(kT2_t[0:D, blk], psA[0:D, 0:256])
            nc.any.tensor_copy(vstage[D:2 * D, blk], psA[D:2 * D, 0:256])

        def kv_projB(c):
            psB = scr.tile([P, 512], f32, tag="scr", name=f"pskB{c}")
            for ec in range(EC):
                nc.tensor.matmul(
                    psB[:, 0:256],
                    lhsT=wqab_t[:, ec * 3 * P + 2 * P: ec * 3 * P + 3 * P],
                    rhs=xkv_v[:, ec:ec + 1, c:c + 1, 1:2, :, :],
                    start=(ec == 0),
                    stop=(ec == EC - 1),
                )
            blk = slice(2 * c * P, (2 * c + 2) * P)
            nc.any.tensor_copy(kT2_t[D:2 * D, blk], psB[D:2 * D, 0:256])
            nc.any.tensor_copy(vstage[0:D, blk], psB[0:D, 0:256])

        def v_trans1(tp):
            pv = scr.tile([P, 512], bf16, tag="scr", name=f"pv{tp}")
            nc.tensor.transpose(
                pv[:, 0:P],
                in_=vstage[:, tp * P:(tp + 1) * P],
                identity=ident_t,
            )
            v1v = v1_t.rearrange("p (k e) -> p k e", e=D + 1)
            nc.any.tensor_copy(v1v[:, 2 * tp:2 * tp + 1, 0:D], pv[:, D:P])
            nc.any.tensor_copy(v1v[:, 2 * tp + 1:2 * tp + 2, 0:D], pv[:, 0:D])

        outp_tiles = {}
        pending = []

        def attn_flush():
            if not pending:
                return
            g, t, pt, w, cs = pending.pop(0)
            lo = g * 512
            outp = outp_tiles[g]
            ka, kb = 2 * t, 2 * t + 1
            nc.tensor.matmul(
                outp[:, cs - lo: 512],
                lhsT=v1_t[:, ka * (D + 1):(ka + 1) * (D + 1)],
                rhs=pt[:, 0:w],
                start=(ka == 0),
                stop=(ka == 8 * g + 7),
            )
            nc.tensor.matmul(
                outp[:, cs - lo: 512],
                lhsT=v1_t[:, kb * (D + 1):(kb + 1) * (D + 1)],
                rhs=pt[:, 512:512 + w],
                start=(kb == 0),
                stop=(kb == 8 * g + 7),
            )
            if kb == 8 * g + 7:
                ob = obpool.tile([D + 1, 512], f32)
                nc.any.tensor_copy(ob[:], outp[:, 0:512])
                nc.sync.dma_start(out=out_d[:, lo: lo + 512], in_=ob[:])

        def attn_pair(g, t):
            lo = g * 512
            if g not in outp_tiles:
                outp_tiles[g] = pout.tile(
                    [D + 1, 512], f32, tag="out", name=f"outp{g}"
                )
            qs = t * P
            cs = max(qs, lo)
            w = lo + 512 - cs
            s = psc.tile([P, 1024], f32, tag="sc", name=f"s{g}_{t}")
            nc.tensor.matmul(
                s[:, 0:w],
                lhsT=kT2_t[0:D, t * P:(t + 1) * P],
                rhs=qT_t[0:D, cs: cs + w],
                start=True,
                stop=True,
            )
            nc.tensor.matmul(
                s[:, 512:512 + w],
                lhsT=kT2_t[D:2 * D, t * P:(t + 1) * P],
                rhs=qT_t[D:2 * D, cs: cs + w],
                start=True,
                stop=True,
            )
            pt = ptpool.tile([P, 1024], bf16)
            nc.scalar.activation(
                pt.rearrange("p (u c) -> p u c", c=512)[:, :, 0:w],
                s.rearrange("p (u c) -> p u c", c=512)[:, :, 0:w],
                func=mybir.ActivationFunctionType.Exp, scale=0.125,
            )
            if cs == qs:
                nc.any.tensor_mul(pt[:, 0:P], pt[:, 0:P], mprev_t)
                nc.any.tensor_mul(pt[:, 512:512 + P], pt[:, 512:512 + P], mlast_t)
            pending.append((g, t, pt, w, cs))
            while len(pending) > 3:
                attn_flush()

        qt_proj(0)
        kv_projA(0)
        kv_projB(0)
        v_trans1(0)
        v_trans1(1)
        attn_pair(0, 0)
        kv_projA(1)
        attn_pair(0, 1)
        kv_projB(1)
        qt_proj_a(1)
        attn_pair(0, 2)
        qt_proj_b(1)
        v_trans1(2)
        attn_pair(0, 3)
        v_trans1(3)
        attn_pair(1, 0)
        kv_projA(2)
        attn_pair(1, 1)
        kv_projB(2)
        qt_proj_a(2)
        attn_pair(1, 2)
        qt_proj_b(2)
        attn_pair(1, 3)
        v_trans1(4)
        attn_pair(1, 4)
        kv_projA(3)
        kv_projB(3)
        attn_pair(1, 5)
        v_trans1(5)
        attn_pair(1, 6)
        v_trans1(6)
        attn_pair(1, 7)
        v_trans1(7)
        attn_pair(2, 0)
        kv_projA(4)
        attn_pair(2, 1)
        kv_projB(4)
        attn_pair(2, 2)
        qt_proj_a(3)
        attn_pair(2, 3)
        qt_proj_b(3)
        attn_pair(2, 4)
        v_trans1(8)
        attn_pair(2, 5)
        kv_projA(5)
        kv_projB(5)
        attn_pair(2, 6)
        attn_pair(2, 7)
        v_trans1(9)
        attn_pair(2, 8)
        v_trans1(10)
        attn_pair(2, 9)
        v_trans1(11)
        attn_pair(2, 10)
        attn_pair(2, 11)
        attn_pair(3, 0)
        kv_projA(6)
        attn_pair(3, 1)
        kv_projB(6)
        attn_pair(3, 2)
        v_trans1(12)
        attn_pair(3, 3)
        kv_projA(7)
        attn_pair(3, 4)
        kv_projB(7)
        attn_pair(3, 5)
        v_trans1(13)
        attn_pair(3, 6)
        v_trans1(14)
        attn_pair(3, 7)
        v_trans1(15)
        for t in range(8, NPAIR):
            attn_pair(3, t)
        while pending:
            attn_flush()

    nc.compile()
    return nc


def _shard_inputs(x, Wq, Wk, Wv):
    x = np.asarray(x, np.float32)
    wq = np.asarray(Wq, np.float32)
    wk = np.asarray(Wk, np.float32)
    wv = np.asarray(Wv, np.float32)
    wqab = np.concatenate([wq, wq, wk, wv, wv, wk], axis=1).astype(BF16)
    ident = np.eye(P, dtype=BF16)
    tri = (np.arange(P)[:, None] <= np.arange(P)[None, :]).astype(BF16)
    ones = np.ones((P, P), BF16)
    zeros = np.zeros((P, P), BF16)
    qidx = {h: np.concatenate([np.arange(P) + (2 * i + h) * P for i in range(NQT)]) for h in (0, 1)}
    in_maps = []
    for c in range(8):
        b, h = c // 2, c % 2
        xT = np.ascontiguousarray(x[b].T).astype(BF16)
        xq = np.ascontiguousarray(xT[:, qidx[h]])
        cmask = np.concatenate(
            [tri if h == 0 else ones, zeros if h == 0 else tri, ident], axis=1
        )
        in_maps.append({
            "xkv": xT,
            "xq": xq,
            "wqab": wqab,
            "cmask": cmask,
        })
    return in_maps


def _unshard(results):
    out = np.zeros((B, T, D), np.float32)
    for c, om in enumerate(results):
        b, h = c // 2, c % 2
        o = np.asarray(om["out"], np.float32)
        on = (o[:D] / o[D:D + 1]).T
        for i in range(NQT):
            out[b, (2 * i + h) * P:(2 * i + h + 1) * P] = on[i * P:(i + 1) * P]
    return out


def kernel(x, Wq, Wk, Wv):
    from concourse import bass_utils

    if "nc" not in _CACHE:
        _CACHE["nc"] = _build_bass()
    nc = _CACHE["nc"]
    in_maps = _shard_inputs(x, Wq, Wk, Wv)
    res = bass_utils.run_bass_kernel_spmd(nc, in_maps, core_ids=list(range(8)))
    _CACHE["last_result"] = res
    return _unshard(res.results)
